# revision 27
# baseline (speedup 1.0000x reference)
"""Trainium2 Bass kernel for a dense transformer block (LN1 -> MHA -> LN2 -> MLP).

Sharding: 8 cores = (batch b in 0..3) x (sequence half in 0..1). Each core
computes the block output for its 1024 query tokens; K/V are computed for the
batch's full 2048 tokens on each core, so there is zero cross-core traffic.

v2 design (cost-model-driven):
- fp8e4 DoubleRow matmuls (0.5 cyc/row, 256-deep contraction) for QKV/Wo
  projections, the scores (32-partition head-band layout via host-permuted
  Wk/Wq columns), and probs @ V.
- ctx computed in [q, dh] orientation with a fused ones-column on V so the
  softmax denominator falls out of the same accumulation chain, then
  PE-transposed (bf16) back to feature-major for Wo.
- exp carries a -3 bias (cancels in normalization) so fp8e4 probs can't
  overflow; ctx is scaled x16 before the fp8 cast and Wo pre-scaled /16.
- MLP stays bf16 for accuracy. 2-stage software pipeline: attention of the
  second 512 queries is emitted interleaved with the MLP of the first 512.
- Elementwise work split across DVE / Act / GpSimd; Gelu/Sqrt (table-based)
  batched so act-table reloads stay rare (Copy/Square/Identity are in every
  table set and interleave freely with Exp).
"""

import sys

if '/opt/trn_rl_repo' not in sys.path:
    sys.path.insert(0, '/opt/trn_rl_repo')

import numpy as np
import ml_dtypes

import concourse.tile as tile
import concourse.mybir as mybir
from concourse import bacc
from concourse.bass import ts
from concourse.bass_utils import run_bass_kernel_spmd

P = 128
F32 = mybir.dt.float32
BF16 = mybir.dt.bfloat16
FP8 = mybir.dt.float8e4
AF = mybir.ActivationFunctionType
ALU = mybir.AluOpType
DRow = mybir.MatmulPerfMode.DoubleRow
EPS = 1e-6
EXP_BIAS = -3.0

B, S, D, H, MLP = 4, 2048, 1024, 16, 4096
N_CORES = 8


def build_bass(T, Q, Dm, Hh, Mlp, n_cores, dbg=False):
    dh = Dm // Hh
    assert dh == 64
    n_dc = Dm // P          # 8 feature chunks
    n_tk = T // P           # 16 key chunks
    n_qc = Q // P           # 8 query chunks
    n_mo = Mlp // P         # 32 mlp chunks
    n_g = Hh // 4           # 4 head quads
    TB = 512
    CHUNKS = [(0, 384), (384, 384), (768, 256)]   # mlp pipeline chunks
    inv_d = 1.0 / Dm

    nc = bacc.Bacc("TRN2", target_bir_lowering=False, debug=False,
                   enable_asserts=False, num_devices=n_cores)

    def din(name, shape, dt):
        return nc.dram_tensor(name, shape, dt, kind="ExternalInput").ap()

    xT_d = din("xT", (Dm, T), F32)
    g1_d, be1_d = din("g1", (Dm,), F32), din("be1", (Dm,), F32)
    g2_d, be2_d = din("g2", (Dm,), F32), din("be2", (Dm,), F32)
    wk8_d = din("wk8", (P, n_dc // 2, 2, Dm), FP8)
    wq8_d = din("wq8", (P, n_dc // 2, 2, Dm), FP8)
    wv8_d = din("wv8", (P, n_dc // 2, 2, Dm), FP8)
    wo8_d = din("wo8", (P, n_dc // 2, 2, Dm), FP8)
    w1_d = din("w1h", (Dm, Mlp), BF16)
    w2_d = din("w2h", (Mlp, Dm), BF16)
    bq_d, bk_d = din("bqp", (Dm,), F32), din("bkp", (Dm,), F32)
    bv_d, bo_d = din("bv", (Dm,), F32), din("bo", (Dm,), F32)
    b1_d, b2_d = din("b1", (Mlp,), F32), din("b2", (Dm,), F32)
    ones_d = din("ones16", (P, 1), BF16)
    ident_d = din("ident16", (P, P), BF16)
    yT_d = nc.dram_tensor("yT", (Dm, Q), F32, kind="ExternalOutput").ap()
    dbg_d = {}
    if dbg:
        for nm, shape, dt in [("dXN", (Dm, T), FP8), ("dKT", (P, n_g, 2, T), FP8),
                              ("dQT", (P, n_g, 2, Q), FP8),
                              ("dVT", (P, n_tk, Hh, dh + 1), FP8),
                              ("dCT", (Dm, Q), FP8), ("dH2", (Dm, Q), BF16),
                              ("dXN2", (Dm, Q), BF16)]:
            dbg_d[nm] = nc.dram_tensor(nm, shape, dt, kind="ExternalOutput").ap()

    with tile.TileContext(nc) as tc:
        with tc.tile_pool(name="const", bufs=1) as constp:
            ones_h = constp.tile([P, 1], BF16)
            nc.sync.dma_start(ones_h[:], ones_d[:, :])
            ident = constp.tile([P, P], BF16)
            nc.sync.dma_start(ident[:], ident_d[:, :])
            eps_t = constp.tile([1, 1], F32)
            nc.vector.memset(eps_t[:], EPS)
            nb3_t = constp.tile([P, 1], F32)
            nc.vector.memset(nb3_t[:], EXP_BIAS)

            def vec_tile(src, n, nm):
                t = constp.tile([P, n], F32, tag=nm, name=nm)
                nc.sync.dma_start(t[:], src.rearrange("(c p) -> p c", p=P))
                return t

            g1_t, be1_t = vec_tile(g1_d, n_dc, "g1"), vec_tile(be1_d, n_dc, "be1")
            g2_t, be2_t = vec_tile(g2_d, n_dc, "g2"), vec_tile(be2_d, n_dc, "be2")
            bq_t, bk_t = vec_tile(bq_d, n_dc, "bq"), vec_tile(bk_d, n_dc, "bk")
            bo_t, b2_t = vec_tile(bo_d, n_dc, "bo"), vec_tile(b2_d, n_dc, "b2")
            b1_t = vec_tile(b1_d, n_mo, "b1")
            bv_row = constp.tile([1, Dm], F32)
            nc.sync.dma_start(bv_row[:, :], bv_d[None, :])
            bv_bc = constp.tile([P, Dm], F32)
            nc.gpsimd.partition_broadcast(bv_bc[:], bv_row[:])

            wo_t = constp.tile([P, n_dc // 2, 2, Dm], FP8, name="wo")
            nc.sync.dma_start(wo_t[:], wo8_d[:, :, :, :])

            with tc.tile_pool(name="p_res", bufs=1) as p_res, \
                 tc.tile_pool(name="p_kv", bufs=1) as p_kv:
                XRAW = p_res.tile([P, n_dc, T], BF16)     # raw x (q-first)
                XQ = XRAW[:, :, 0:Q]                      # raw xq, then h2
                KT = p_kv.tile([P, n_g, 2, T], FP8)       # band layout
                QT = p_kv.tile([P, n_g, 2, Q], FP8)
                VT = p_kv.tile([P, n_tk, Hh, dh + 1], FP8)
                nc.gpsimd.memset(VT[:, :, :, dh:dh + 1], 1.0)

                # ---------- Phase A/B : LN1 + QKV projections ----------
                with tc.tile_pool(name="p_act", bufs=1) as p_act, \
                     tc.tile_pool(name="p_tmp", bufs=6) as p_tmp, \
                     tc.tile_pool(name="p_st", bufs=1) as p_st, \
                     tc.tile_pool(name="ps_st", bufs=2, space="PSUM") as ps_st, \
                     tc.tile_pool(name="ps_pr", bufs=6, space="PSUM") as ps_pr:

                    XN = p_act.tile([P, n_dc, T], FP8)
                    XNQ = XN[:, :, 0:Q]
                    wk_t = p_act.tile([P, n_dc // 2, 2, Dm], FP8, name="wk")
                    nc.sync.dma_start(wk_t[:], wk8_d[:, :, :, :])
                    wq_t = p_act.tile([P, n_dc // 2, 2, Dm], FP8, name="wq")
                    nc.sync.dma_start(wq_t[:], wq8_d[:, :, :, :])
                    wv_t = p_act.tile([P, n_dc // 2, 2, Dm], FP8, name="wv")
                    nc.sync.dma_start(wv_t[:], wv8_d[:, :, :, :])

                    def ln_block(n_tok, xdram, xraw, g_t, b_t, out_ap,
                                 preloaded):
                        for tb in range(n_tok // TB):
                            sl = ts(tb, TB)
                            if not preloaded:
                                for dc in range(n_dc):
                                    xc = p_tmp.tile([P, TB], F32, tag="xc",
                                                    name="xc")
                                    nc.sync.dma_start(xc[:],
                                                      xdram[ts(dc, P), sl])
                                    nc.gpsimd.tensor_copy(xraw[:, dc, sl],
                                                          xc[:])
                            ps_m = ps_st.tile([1, TB], F32, tag="ps_st",
                                              name="ps_m")
                            for dc in range(n_dc):
                                nc.tensor.matmul(ps_m[:], ones_h[:],
                                                 xraw[:, dc, sl],
                                                 start=(dc == 0),
                                                 stop=(dc == n_dc - 1))
                            sq = p_tmp.tile([P, n_dc, TB], BF16, tag="sq",
                                            name="sq", bufs=1)
                            for dc in range(n_dc):
                                nc.vector.tensor_tensor(
                                    sq[:, dc, :], xraw[:, dc, sl],
                                    xraw[:, dc, sl], ALU.mult)
                            ps_s = ps_st.tile([1, TB], F32, tag="ps_st",
                                              name="ps_s")
                            for dc in range(n_dc):
                                nc.tensor.matmul(ps_s[:], ones_h[:],
                                                 sq[:, dc, :],
                                                 start=(dc == 0),
                                                 stop=(dc == n_dc - 1))
                            mean = p_st.tile([1, TB], F32, tag="mean",
                                             name="mean")
                            nc.vector.tensor_scalar_mul(mean[:], ps_m[:],
                                                        inv_d)
                            ex2 = p_st.tile([1, TB], F32, tag="ex2",
                                            name="ex2")
                            nc.vector.tensor_scalar_mul(ex2[:], ps_s[:],
                                                        inv_d)
                            var = p_st.tile([1, TB], F32, tag="var",
                                            name="var")
                            nc.vector.tensor_tensor(var[:], mean[:], mean[:],
                                                    ALU.mult)
                            nc.vector.tensor_tensor(var[:], ex2[:], var[:],
                                                    ALU.subtract)
                            std = p_st.tile([1, TB], F32, tag="std",
                                            name="std")
                            nc.scalar.activation(std[:], var[:], AF.Sqrt,
                                                 bias=eps_t[:, :])
                            rstd = p_st.tile([1, TB], F32, tag="rstd",
                                             name="rstd")
                            nc.vector.reciprocal(rstd[:], std[:])
                            mean_h = p_st.tile([1, TB], BF16, tag="mean_h",
                                               name="mean_h", bufs=2)
                            nc.vector.tensor_copy(mean_h[:], mean[:])
                            rstd_h = p_st.tile([1, TB], BF16, tag="rstd_h",
                                               name="rstd_h", bufs=2)
                            nc.vector.tensor_copy(rstd_h[:], rstd[:])
                            mbc = p_tmp.tile([P, TB], BF16, tag="mbc",
                                             name="mbc", bufs=2)
                            nc.gpsimd.partition_broadcast(mbc[:], mean_h[:])
                            rbc = p_tmp.tile([P, TB], BF16, tag="rbc",
                                             name="rbc", bufs=2)
                            nc.gpsimd.partition_broadcast(rbc[:], rstd_h[:])
                            for dc in range(n_dc):
                                t0 = p_tmp.tile([P, TB], BF16, tag="t0",
                                                name="t0")
                                nc.vector.tensor_tensor(
                                    t0[:], xraw[:, dc, sl], mbc[:],
                                    ALU.subtract)
                                nc.vector.tensor_tensor(t0[:], t0[:], rbc[:],
                                                        ALU.mult)
                                nc.scalar.activation(
                                    out_ap[:, dc, sl], t0[:], AF.Identity,
                                    bias=b_t[:, dc:dc + 1],
                                    scale=g_t[:, dc:dc + 1])

                    ln_block(T, xT_d, XRAW, g1_t, be1_t, XN, False)

                    def kq_proj(w_t, b_t_, out, n_tok, src):
                        for g in range(n_g):
                            for t in range(2):
                                m = 2 * g + t
                                for tb in range(n_tok // TB):
                                    ps = ps_pr.tile([P, TB], F32, tag="ps_pr",
                                                    name="ps_pr")
                                    for p_ in range(n_dc // 2):
                                        nc.tensor.matmul(
                                            ps[:],
                                            w_t[:, p_, :, ts(m, P)],
                                            src[:, 2 * p_:2 * p_ + 2,
                                                ts(tb, TB)],
                                            start=(p_ == 0),
                                            stop=(p_ == n_dc // 2 - 1),
                                            perf_mode=DRow)
                                    nc.scalar.activation(
                                        out[:, g, t, ts(tb, TB)], ps[:],
                                        AF.Identity, bias=b_t_[:, m:m + 1])
                    kq_proj(wk_t, bk_t, KT, T, XN)
                    kq_proj(wq_t, bq_t, QT, Q, XNQ)

                    NO = 512
                    for to in range(n_tk):
                        for no in range(Dm // NO):
                            ps = ps_pr.tile([P, NO], F32, tag="ps_pr",
                                            name="ps_pr")
                            for p_ in range(n_dc // 2):
                                nc.tensor.matmul(
                                    ps[:],
                                    XN[:, 2 * p_:2 * p_ + 2, ts(to, P)],
                                    wv_t[:, p_, :, ts(no, NO)],
                                    start=(p_ == 0),
                                    stop=(p_ == n_dc // 2 - 1),
                                    perf_mode=DRow)
                            h0 = no * (NO // dh)
                            nc.vector.tensor_tensor(
                                VT[:, to, h0:h0 + NO // dh, 0:dh],
                                ps[:].rearrange("p (h e) -> p h e", e=dh),
                                bv_bc[:, ts(no, NO)]
                                .rearrange("p (h e) -> p h e", e=dh),
                                ALU.add)

                    if dbg:
                        for dc in range(n_dc):
                            nc.sync.dma_start(dbg_d["dXN"][ts(dc, P), :],
                                              XN[:, dc, :])
                        nc.sync.dma_start(dbg_d["dKT"][:, :, :, :], KT[:])
                        nc.sync.dma_start(dbg_d["dQT"][:, :, :, :], QT[:])
                        nc.sync.dma_start(dbg_d["dVT"][:, :, :, :], VT[:])

                # ---------- Phase C/D : attention pipelined with MLP ----------
                with tc.tile_pool(name="p_exps", bufs=3) as p_exps, \
                     tc.tile_pool(name="p_ctm", bufs=2) as p_ctm, \
                     tc.tile_pool(name="p_ct", bufs=1) as p_ct, \
                     tc.tile_pool(name="p_mlp", bufs=1) as p_mlp, \
                     tc.tile_pool(name="p_w1", bufs=4) as p_w1, \
                     tc.tile_pool(name="p_w2", bufs=4) as p_w2, \
                     tc.tile_pool(name="p_st2", bufs=2) as p_st2, \
                     tc.tile_pool(name="p_tm2", bufs=2) as p_tm2, \
                     tc.tile_pool(name="p_out", bufs=2) as p_out, \
                     tc.tile_pool(name="ps_sc", bufs=2, space="PSUM") as ps_sc, \
                     tc.tile_pool(name="ps_cx", bufs=1, space="PSUM") as ps_cx, \
                     tc.tile_pool(name="ps_mx", bufs=2, space="PSUM") as ps_mx, \
                     tc.tile_pool(name="ps_s2", bufs=1, space="PSUM") as ps_s2p:

                    CT = p_ct.tile([P, n_dc, Q], FP8)
                    XN2 = p_mlp.tile([P, n_dc, Q], BF16)
                    Y1 = {}
                    cx_cur = [None]
                    ctm_cur = [None]

                    def attn_unit(qc, h):
                        g, b4 = h // 4, h % 4
                        r0 = 32 * b4
                        qsl = ts(qc, P)
                        if h % 8 == 0:
                            ctm_cur[0] = p_ctm.tile([P, 8, dh], BF16,
                                                    tag="ctm", name="ctm")
                        if h % 4 == 0:
                            cx_cur[0] = ps_cx.tile([P, 4, P], F32,
                                                   tag="ps_cx", name="ps_cx")
                        exps = p_exps.tile([P, n_tk, P], FP8, tag="exps",
                                           name="exps")
                        for half in range(2):
                            pss = ps_sc.tile([P, n_tk // 2, P], F32,
                                             tag="ps_sc", name="ps_sc")
                            for i in range(n_tk // 2):
                                kc = half * (n_tk // 2) + i
                                nc.tensor.matmul(
                                    pss[:, i, :],
                                    KT[r0:r0 + 32, g, :, ts(kc, P)],
                                    QT[r0:r0 + 32, g, :, qsl],
                                    start=True, stop=True, perf_mode=DRow,
                                    tile_position=(r0, 0))
                            nc.scalar.activation(
                                exps[:, half * (n_tk // 2):
                                     (half + 1) * (n_tk // 2), :],
                                pss[:, :, :], AF.Exp, scale=0.125,
                                bias=nb3_t[:, :])
                        ps_c = cx_cur[0]
                        for i in range(n_tk // 2):
                            nc.tensor.matmul(
                                ps_c[:, h % 4, 0:dh + 1],
                                exps[:, 2 * i:2 * i + 2, :],
                                VT[:, 2 * i:2 * i + 2, h, :],
                                start=(i == 0), stop=(i == n_tk // 2 - 1),
                                perf_mode=DRow)
                        if h % 4 == 3:
                            grp = h // 4
                            ctm = ctm_cur[0]
                            rb = p_st2.tile([P, 4, 1], F32, tag="rb",
                                            name="rb")
                            nc.vector.reciprocal(rb[:],
                                                 ps_c[:, :, dh:dh + 1])
                            nc.vector.tensor_tensor(
                                ctm[:, (grp % 2) * 4:(grp % 2) * 4 + 4, :],
                                ps_c[:, :, 0:dh],
                                rb[:].broadcast_to((P, 4, dh)), ALU.mult)
                            if grp % 2 == 1:
                                for j in range(4):
                                    dc = (grp // 2) * 4 + j
                                    tr = ps_mx.tile([P, P], BF16,
                                                    tag="ps_mx", name="tr")
                                    nc.tensor.transpose(
                                        tr[:], ctm[:, 2 * j:2 * j + 2, :],
                                        ident[:])
                                    nc.vector.tensor_copy(
                                        CT[:, dc, ts(qc, P)], tr[:])

                    def emit_attn(qc):
                        for h in range(Hh):
                            attn_unit(qc, h)

                    def emit_wo_ln2(c):
                        tok0, CH = CHUNKS[c]
                        csl = slice(tok0, tok0 + CH)
                        for m in range(n_dc):
                            ps = ps_mx.tile([P, CH], F32, tag="ps_mx",
                                            name="ps_mo")
                            for p_ in range(n_dc // 2):
                                nc.tensor.matmul(
                                    ps[:], wo_t[:, p_, :, ts(m, P)],
                                    CT[:, 2 * p_:2 * p_ + 2, csl],
                                    start=(p_ == 0),
                                    stop=(p_ == n_dc // 2 - 1),
                                    perf_mode=DRow)
                            t1 = p_tm2.tile([P, CH], BF16, tag="t1",
                                            name="t1")
                            nc.vector.tensor_tensor(t1[:], ps[:],
                                                    XQ[:, m, csl], ALU.add)
                            nc.gpsimd.tensor_scalar_add(XQ[:, m, csl], t1[:],
                                                        bo_t[:, m:m + 1])
                        # LN2 on the chunk
                        ps_m = ps_s2p.tile([1, CH], F32, tag="ps_s2",
                                           name="ps_m2")
                        for dc in range(n_dc):
                            nc.tensor.matmul(ps_m[:], ones_h[:],
                                             XQ[:, dc, csl],
                                             start=(dc == 0),
                                             stop=(dc == n_dc - 1))
                        mean = p_st2.tile([1, CH], F32, tag="mean2",
                                          name="mean2")
                        nc.vector.tensor_scalar_mul(mean[:], ps_m[:], inv_d)
                        ps_s = ps_s2p.tile([1, CH], F32, tag="ps_s2",
                                           name="ps_s2")
                        for dc in range(n_dc):
                            sq = p_tm2.tile([P, CH], BF16, tag="sq2",
                                            name="sq2", bufs=1)
                            nc.vector.tensor_tensor(sq[:],
                                                    XQ[:, dc, csl],
                                                    XQ[:, dc, csl], ALU.mult)
                            nc.tensor.matmul(ps_s[:], ones_h[:], sq[:],
                                             start=(dc == 0),
                                             stop=(dc == n_dc - 1))
                        ex2 = p_st2.tile([1, CH], F32, tag="ex22",
                                         name="ex22")
                        nc.vector.tensor_scalar_mul(ex2[:], ps_s[:], inv_d)
                        var = p_st2.tile([1, CH], F32, tag="var2",
                                         name="var2")
                        nc.vector.tensor_tensor(var[:], mean[:], mean[:],
                                                ALU.mult)
                        nc.vector.tensor_tensor(var[:], ex2[:], var[:],
                                                ALU.subtract)
                        std = p_st2.tile([1, CH], F32, tag="std2",
                                         name="std2")
                        nc.scalar.activation(std[:], var[:], AF.Sqrt,
                                             bias=eps_t[:, :])
                        rstd = p_st2.tile([1, CH], F32, tag="rstd2",
                                          name="rstd2")
                        nc.vector.reciprocal(rstd[:], std[:])
                        mean_h = p_st2.tile([1, CH], BF16, tag="mean2h",
                                            name="mean2h")
                        nc.vector.tensor_copy(mean_h[:], mean[:])
                        rstd_h = p_st2.tile([1, CH], BF16, tag="rstd2h",
                                            name="rstd2h")
                        nc.vector.tensor_copy(rstd_h[:], rstd[:])
                        mbc = p_tm2.tile([P, CH], BF16, tag="mbc2",
                                         name="mbc2", bufs=1)
                        nc.gpsimd.partition_broadcast(mbc[:], mean_h[:])
                        rbc = p_tm2.tile([P, CH], BF16, tag="rbc2",
                                         name="rbc2", bufs=1)
                        nc.gpsimd.partition_broadcast(rbc[:], rstd_h[:])
                        for dc in range(n_dc):
                            t0 = p_tm2.tile([P, CH], BF16, tag="t02",
                                            name="t02")
                            nc.vector.tensor_tensor(t0[:], XQ[:, dc, csl],
                                                    mbc[:], ALU.subtract)
                            nc.vector.tensor_tensor(t0[:], t0[:], rbc[:],
                                                    ALU.mult)
                            nc.vector.tensor_scalar(
                                XN2[:, dc, csl], t0[:], g2_t[:, dc:dc + 1],
                                be2_t[:, dc:dc + 1], ALU.mult, ALU.add)

                    def emit_fc1(c, mo0, mo1):
                        tok0, CH = CHUNKS[c]
                        csl = slice(tok0, tok0 + CH)
                        if mo0 == 0:
                            Y1[c] = p_mlp.tile([P, n_mo, CHUNKS[c][1]], BF16,
                                               tag="y1", name="y1", bufs=1,
                                               padded_shape=[P, n_mo, 384])
                        for mo in range(mo0, mo1):
                            wt = p_w1.tile([P, n_dc, P], BF16, tag="w1",
                                           name="w1")
                            nc.sync.dma_start(
                                wt[:],
                                w1_d[:, ts(mo, P)]
                                .rearrange("(c p) m -> p c m", p=P))
                            ps = ps_mx.tile([P, CH], F32, tag="ps_mx",
                                            name="ps_f1")
                            for dc in range(n_dc):
                                nc.tensor.matmul(ps[:], wt[:, dc, :],
                                                 XN2[:, dc, csl],
                                                 start=(dc == 0),
                                                 stop=(dc == n_dc - 1))
                            nc.vector.tensor_scalar_add(Y1[c][:, mo, :],
                                                        ps[:],
                                                        b1_t[:, mo:mo + 1])

                    def emit_gelu(c):
                        for g8 in range(0, n_mo, 16):
                            nc.scalar.activation(Y1[c][:, g8:g8 + 16, :],
                                                 Y1[c][:, g8:g8 + 16, :],
                                                 AF.Gelu)

                    def emit_fc2(c):
                        tok0, CH = CHUNKS[c]
                        csl = slice(tok0, tok0 + CH)
                        for m2 in range(n_dc):
                            wt = p_w2.tile([P, n_mo // 2, P], BF16, tag="w2",
                                           name="w2")
                            nc.sync.dma_start(
                                wt[:],
                                w2_d[0:Mlp // 2, ts(m2, P)]
                                .rearrange("(c p) m -> p c m", p=P))
                            wt2 = p_w2.tile([P, n_mo // 2, P], BF16, tag="w2",
                                            name="w2b")
                            nc.sync.dma_start(
                                wt2[:],
                                w2_d[Mlp // 2:Mlp, ts(m2, P)]
                                .rearrange("(c p) m -> p c m", p=P))
                            ps = ps_mx.tile([P, CH], F32, tag="ps_mx",
                                            name="ps_f2")
                            for kc in range(n_mo):
                                w_ = wt if kc < n_mo // 2 else wt2
                                nc.tensor.matmul(ps[:],
                                                 w_[:, kc % (n_mo // 2), :],
                                                 Y1[c][:, kc, :],
                                                 start=(kc == 0),
                                                 stop=(kc == n_mo - 1))
                            ot = p_out.tile([P, CH], F32, tag="out",
                                            name="out")
                            nc.vector.tensor_tensor(ot[:], ps[:],
                                                    XQ[:, m2, csl], ALU.add)
                            nc.gpsimd.tensor_scalar_add(ot[:], ot[:],
                                                        b2_t[:, m2:m2 + 1])
                            nc.sync.dma_start(yT_d[ts(m2, P), csl], ot[:])

                    # 3-chunk software pipeline: MLP of chunk c emitted
                    # under the attention of later query chunks.
                    emit_attn(0)
                    emit_attn(1)
                    emit_attn(2)
                    emit_wo_ln2(0)
                    emit_attn(3)
                    emit_fc1(0, 0, 32)
                    emit_attn(4)
                    emit_gelu(0)
                    emit_fc2(0)
                    emit_wo_ln2(1)
                    emit_attn(5)
                    emit_fc1(1, 0, 16)
                    emit_attn(6)
                    emit_fc1(1, 16, 32)
                    emit_attn(7)
                    emit_gelu(1)
                    emit_fc2(1)
                    emit_wo_ln2(2)
                    emit_fc1(2, 0, 32)
                    emit_gelu(2)
                    emit_fc2(2)
                    if dbg:
                        for dc in range(n_dc):
                            nc.sync.dma_start(dbg_d["dCT"][ts(dc, P), :],
                                              CT[:, dc, :])
                            nc.sync.dma_start(dbg_d["dH2"][ts(dc, P), :],
                                              XQ[:, dc, :])
                            nc.sync.dma_start(dbg_d["dXN2"][ts(dc, P), :],
                                              XN2[:, dc, :])
    nc.compile()
    return nc


_NC_CACHE = {}


def _get_nc(T, Q, Dm, Hh, Mlp, n_cores):
    key = (T, Q, Dm, Hh, Mlp, n_cores)
    if key not in _NC_CACHE:
        _NC_CACHE[key] = build_bass(T, Q, Dm, Hh, Mlp, n_cores)
    return _NC_CACHE[key]


def _perm_cols(Dm):
    """Column permutation for the K/Q DoubleRow band layout.
    Chunk m=2g+t, partition p=32*b4+r  ->  original feature
    (4g+b4)*64 + t*32 + r."""
    perm = np.empty(Dm, np.int64)
    for m in range(Dm // P):
        g, t = m // 2, m % 2
        for p in range(P):
            b4, r = p // 32, p % 32
            perm[m * P + p] = (4 * g + b4) * 64 + t * 32 + r
    return perm


def _dr_weight(w, scale=1.0):
    """[D, N] -> (128, D/256, 2, N) fp8: [r, p, t, c] = w[(2p+t)*128+r, c]."""
    Dm = w.shape[0]
    f8 = ml_dtypes.float8_e4m3
    return np.ascontiguousarray(
        (w.reshape(Dm // 256, 2, P, w.shape[1]) * scale)
        .transpose(2, 0, 1, 3)).astype(f8)


def make_in_maps(inputs, n_cores):
    x = np.asarray(inputs["x"], np.float32)
    Bq, Sq, Dq = x.shape
    Qtok = Sq * Bq // n_cores
    bf = ml_dtypes.bfloat16
    perm = _perm_cols(Dq)
    wk = np.asarray(inputs["Wk"], np.float32)[:, perm]
    wq = np.asarray(inputs["Wq"], np.float32)[:, perm]
    shared = {
        "g1": np.asarray(inputs["ln1_g"], np.float32),
        "be1": np.asarray(inputs["ln1_b"], np.float32),
        "g2": np.asarray(inputs["ln2_g"], np.float32),
        "be2": np.asarray(inputs["ln2_b"], np.float32),
        "wk8": _dr_weight(wk),
        "wq8": _dr_weight(wq),
        "wv8": _dr_weight(np.asarray(inputs["Wv"], np.float32)),
        "wo8": _dr_weight(np.asarray(inputs["Wo"], np.float32)),
        "w1h": np.asarray(inputs["W1"], np.float32).astype(bf),
        "w2h": np.asarray(inputs["W2"], np.float32).astype(bf),
        "bkp": np.asarray(inputs["bk"], np.float32)[perm],
        "bqp": np.asarray(inputs["bq"], np.float32)[perm],
        "bv": np.asarray(inputs["bv"], np.float32),
        "bo": np.asarray(inputs["bo"], np.float32),
        "b1": np.asarray(inputs["b1"], np.float32),
        "b2": np.asarray(inputs["b2"], np.float32),
        "ones16": np.ones((P, 1), bf),
        "ident16": np.eye(P, dtype=bf),
    }
    in_maps = []
    for c in range(n_cores):
        b = c // (n_cores // Bq)
        qoff = (c % (n_cores // Bq)) * Qtok
        m = dict(shared)
        xrot = np.concatenate([x[b, qoff:], x[b, :qoff]], axis=0)
        m["xT"] = np.ascontiguousarray(xrot.T)
        in_maps.append(m)
    return in_maps, Qtok


def kernel(**inputs):
    x = np.asarray(inputs["x"], np.float32)
    Bq, Sq, Dq = x.shape
    in_maps, Qtok = make_in_maps(inputs, N_CORES)
    nc = _get_nc(Sq, Qtok, Dq, H, MLP, N_CORES)
    res = run_bass_kernel_spmd(nc, in_maps, core_ids=list(range(N_CORES)))
    out = np.empty((Bq, Sq, Dq), np.float32)
    per_b = N_CORES // Bq
    for c in range(N_CORES):
        b = c // per_b
        qoff = (c % per_b) * Qtok
        out[b, qoff:qoff + Qtok, :] = res.results[c]["yT"].T
    return out


# revision 28
# speedup vs baseline: 1.0593x; 1.0593x over previous
"""Trainium2 Bass kernel for a dense transformer block (LN1 -> MHA -> LN2 -> MLP).

Sharding: 8 cores = (batch b in 0..3) x (sequence half in 0..1). Each core
computes the block output for its 1024 query tokens; K/V are computed for the
batch's full 2048 tokens on each core, so there is zero cross-core traffic.

v2 design (cost-model-driven):
- fp8e4 DoubleRow matmuls (0.5 cyc/row, 256-deep contraction) for QKV/Wo
  projections, the scores (32-partition head-band layout via host-permuted
  Wk/Wq columns), and probs @ V.
- ctx computed in [q, dh] orientation with a fused ones-column on V so the
  softmax denominator falls out of the same accumulation chain, then
  PE-transposed (bf16) back to feature-major for Wo.
- exp carries a -3 bias (cancels in normalization) so fp8e4 probs can't
  overflow; ctx is scaled x16 before the fp8 cast and Wo pre-scaled /16.
- MLP stays bf16 for accuracy. 2-stage software pipeline: attention of the
  second 512 queries is emitted interleaved with the MLP of the first 512.
- Elementwise work split across DVE / Act / GpSimd; Gelu/Sqrt (table-based)
  batched so act-table reloads stay rare (Copy/Square/Identity are in every
  table set and interleave freely with Exp).
"""

import sys

if '/opt/trn_rl_repo' not in sys.path:
    sys.path.insert(0, '/opt/trn_rl_repo')

import numpy as np
import ml_dtypes

import concourse.tile as tile
import concourse.mybir as mybir
from concourse import bacc
from concourse.bass import ts
from concourse.bass_utils import run_bass_kernel_spmd

P = 128
F32 = mybir.dt.float32
BF16 = mybir.dt.bfloat16
FP8 = mybir.dt.float8e4
AF = mybir.ActivationFunctionType
ALU = mybir.AluOpType
DRow = mybir.MatmulPerfMode.DoubleRow
EPS = 1e-6
EXP_BIAS = -3.0

B, S, D, H, MLP = 4, 2048, 1024, 16, 4096
N_CORES = 8


def build_bass(T, Q, Dm, Hh, Mlp, n_cores, dbg=False):
    dh = Dm // Hh
    assert dh == 64
    n_dc = Dm // P          # 8 feature chunks
    n_tk = T // P           # 16 key chunks
    n_qc = Q // P           # 8 query chunks
    n_mo = Mlp // P         # 32 mlp chunks
    n_g = Hh // 4           # 4 head quads
    TB = 512
    CHUNKS = [(0, 256), (256, 384), (640, 384)]   # mlp pipeline chunks
    inv_d = 1.0 / Dm

    nc = bacc.Bacc("TRN2", target_bir_lowering=False, debug=False,
                   enable_asserts=False, num_devices=n_cores)

    def din(name, shape, dt):
        return nc.dram_tensor(name, shape, dt, kind="ExternalInput").ap()

    xT_d = din("xT", (Dm, T), F32)
    g1_d, be1_d = din("g1", (Dm,), F32), din("be1", (Dm,), F32)
    g2_d, be2_d = din("g2", (Dm,), F32), din("be2", (Dm,), F32)
    wk8_d = din("wk8", (P, n_dc // 2, 2, Dm), FP8)
    wq8_d = din("wq8", (P, n_dc // 2, 2, Dm), FP8)
    wv8_d = din("wv8", (P, n_dc // 2, 2, Dm), FP8)
    wo8_d = din("wo8", (P, n_dc // 2, 2, Dm), FP8)
    w1_d = din("w1h", (Dm, Mlp), BF16)
    w2_d = din("w2h", (Mlp, Dm), BF16)
    bq_d, bk_d = din("bqp", (Dm,), F32), din("bkp", (Dm,), F32)
    bv_d, bo_d = din("bv", (Dm,), F32), din("bo", (Dm,), F32)
    b1_d, b2_d = din("b1", (Mlp,), F32), din("b2", (Dm,), F32)
    ones_d = din("ones16", (P, 1), BF16)
    ident_d = din("ident16", (P, P), BF16)
    yT_d = nc.dram_tensor("yT", (Dm, Q), F32, kind="ExternalOutput").ap()
    dbg_d = {}
    if dbg:
        for nm, shape, dt in [("dXN", (Dm, T), FP8), ("dKT", (P, n_g, 2, T), FP8),
                              ("dQT", (P, n_g, 2, Q), FP8),
                              ("dVT", (P, n_tk, Hh, dh + 1), FP8),
                              ("dCT", (Dm, Q), FP8), ("dH2", (Dm, Q), BF16),
                              ("dXN2", (Dm, Q), BF16)]:
            dbg_d[nm] = nc.dram_tensor(nm, shape, dt, kind="ExternalOutput").ap()

    with tile.TileContext(nc) as tc:
        with tc.tile_pool(name="const", bufs=1) as constp:
            ones_h = constp.tile([P, 1], BF16)
            nc.sync.dma_start(ones_h[:], ones_d[:, :])
            ident = constp.tile([P, P], BF16)
            nc.sync.dma_start(ident[:], ident_d[:, :])
            eps_t = constp.tile([1, 1], F32)
            nc.vector.memset(eps_t[:], EPS)
            nb3_t = constp.tile([P, 1], F32)
            nc.vector.memset(nb3_t[:], EXP_BIAS)

            def vec_tile(src, n, nm):
                t = constp.tile([P, n], F32, tag=nm, name=nm)
                nc.sync.dma_start(t[:], src.rearrange("(c p) -> p c", p=P))
                return t

            g1_t, be1_t = vec_tile(g1_d, n_dc, "g1"), vec_tile(be1_d, n_dc, "be1")
            g2_t, be2_t = vec_tile(g2_d, n_dc, "g2"), vec_tile(be2_d, n_dc, "be2")
            bq_t, bk_t = vec_tile(bq_d, n_dc, "bq"), vec_tile(bk_d, n_dc, "bk")
            bo_t, b2_t = vec_tile(bo_d, n_dc, "bo"), vec_tile(b2_d, n_dc, "b2")
            b1_t = vec_tile(b1_d, n_mo, "b1")
            bv_row = constp.tile([1, Dm], F32)
            nc.sync.dma_start(bv_row[:, :], bv_d[None, :])
            bv_bc = constp.tile([P, Dm], F32)
            nc.gpsimd.partition_broadcast(bv_bc[:], bv_row[:])

            wo_t = constp.tile([P, n_dc // 2, 2, Dm], FP8, name="wo")
            nc.sync.dma_start(wo_t[:], wo8_d[:, :, :, :])

            with tc.tile_pool(name="p_res", bufs=1) as p_res, \
                 tc.tile_pool(name="p_kv", bufs=1) as p_kv:
                XRAW = p_res.tile([P, n_dc, T], BF16)     # raw x (q-first)
                XQ = XRAW[:, :, 0:Q]                      # raw xq, then h2
                KT = p_kv.tile([P, n_g, 2, T], FP8)       # band layout
                QT = p_kv.tile([P, n_g, 2, Q], FP8)
                VT = p_kv.tile([P, n_tk, Hh, dh + 1], FP8)
                nc.gpsimd.memset(VT[:, :, :, dh:dh + 1], 1.0)

                # ---------- Phase A/B : LN1 + QKV projections ----------
                with tc.tile_pool(name="p_act", bufs=1) as p_act, \
                     tc.tile_pool(name="p_tmp", bufs=6) as p_tmp, \
                     tc.tile_pool(name="p_st", bufs=1) as p_st, \
                     tc.tile_pool(name="ps_st", bufs=2, space="PSUM") as ps_st, \
                     tc.tile_pool(name="ps_pr", bufs=6, space="PSUM") as ps_pr:

                    XN = p_act.tile([P, n_dc, T], FP8)
                    XNQ = XN[:, :, 0:Q]
                    wk_t = p_act.tile([P, n_dc // 2, 2, Dm], FP8, name="wk")
                    nc.sync.dma_start(wk_t[:], wk8_d[:, :, :, :])
                    wq_t = p_act.tile([P, n_dc // 2, 2, Dm], FP8, name="wq")
                    nc.sync.dma_start(wq_t[:], wq8_d[:, :, :, :])
                    wv_t = p_act.tile([P, n_dc // 2, 2, Dm], FP8, name="wv")
                    nc.sync.dma_start(wv_t[:], wv8_d[:, :, :, :])

                    def ln_block(n_tok, xdram, xraw, g_t, b_t, out_ap,
                                 preloaded):
                        for tb in range(n_tok // TB):
                            sl = ts(tb, TB)
                            if not preloaded:
                                for dc in range(n_dc):
                                    xc = p_tmp.tile([P, TB], F32, tag="xc",
                                                    name="xc")
                                    nc.sync.dma_start(xc[:],
                                                      xdram[ts(dc, P), sl])
                                    nc.gpsimd.tensor_copy(xraw[:, dc, sl],
                                                          xc[:])
                            ps_m = ps_st.tile([1, TB], F32, tag="ps_st",
                                              name="ps_m")
                            for dc in range(n_dc):
                                nc.tensor.matmul(ps_m[:], ones_h[:],
                                                 xraw[:, dc, sl],
                                                 start=(dc == 0),
                                                 stop=(dc == n_dc - 1))
                            sq = p_tmp.tile([P, n_dc, TB], BF16, tag="sq",
                                            name="sq", bufs=1)
                            for dc in range(n_dc):
                                nc.vector.tensor_tensor(
                                    sq[:, dc, :], xraw[:, dc, sl],
                                    xraw[:, dc, sl], ALU.mult)
                            ps_s = ps_st.tile([1, TB], F32, tag="ps_st",
                                              name="ps_s")
                            for dc in range(n_dc):
                                nc.tensor.matmul(ps_s[:], ones_h[:],
                                                 sq[:, dc, :],
                                                 start=(dc == 0),
                                                 stop=(dc == n_dc - 1))
                            mean = p_st.tile([1, TB], F32, tag="mean",
                                             name="mean")
                            nc.vector.tensor_scalar_mul(mean[:], ps_m[:],
                                                        inv_d)
                            ex2 = p_st.tile([1, TB], F32, tag="ex2",
                                            name="ex2")
                            nc.vector.tensor_scalar_mul(ex2[:], ps_s[:],
                                                        inv_d)
                            var = p_st.tile([1, TB], F32, tag="var",
                                            name="var")
                            nc.vector.tensor_tensor(var[:], mean[:], mean[:],
                                                    ALU.mult)
                            nc.vector.tensor_tensor(var[:], ex2[:], var[:],
                                                    ALU.subtract)
                            std = p_st.tile([1, TB], F32, tag="std",
                                            name="std")
                            nc.scalar.activation(std[:], var[:], AF.Sqrt,
                                                 bias=eps_t[:, :])
                            rstd = p_st.tile([1, TB], F32, tag="rstd",
                                             name="rstd")
                            nc.vector.reciprocal(rstd[:], std[:])
                            mean_h = p_st.tile([1, TB], BF16, tag="mean_h",
                                               name="mean_h", bufs=2)
                            nc.vector.tensor_copy(mean_h[:], mean[:])
                            rstd_h = p_st.tile([1, TB], BF16, tag="rstd_h",
                                               name="rstd_h", bufs=2)
                            nc.vector.tensor_copy(rstd_h[:], rstd[:])
                            mbc = p_tmp.tile([P, TB], BF16, tag="mbc",
                                             name="mbc", bufs=2)
                            nc.gpsimd.partition_broadcast(mbc[:], mean_h[:])
                            rbc = p_tmp.tile([P, TB], BF16, tag="rbc",
                                             name="rbc", bufs=2)
                            nc.gpsimd.partition_broadcast(rbc[:], rstd_h[:])
                            for dc in range(n_dc):
                                t0 = p_tmp.tile([P, TB], BF16, tag="t0",
                                                name="t0")
                                nc.vector.tensor_tensor(
                                    t0[:], xraw[:, dc, sl], mbc[:],
                                    ALU.subtract)
                                nc.vector.tensor_tensor(t0[:], t0[:], rbc[:],
                                                        ALU.mult)
                                nc.scalar.activation(
                                    out_ap[:, dc, sl], t0[:], AF.Identity,
                                    bias=b_t[:, dc:dc + 1],
                                    scale=g_t[:, dc:dc + 1])

                    ln_block(T, xT_d, XRAW, g1_t, be1_t, XN, False)

                    def kq_proj(w_t, b_t_, out, n_tok, src, on_dve=False):
                        for g in range(n_g):
                            for t in range(2):
                                m = 2 * g + t
                                for tb in range(n_tok // TB):
                                    ps = ps_pr.tile([P, TB], F32, tag="ps_pr",
                                                    name="ps_pr")
                                    for p_ in range(n_dc // 2):
                                        nc.tensor.matmul(
                                            ps[:],
                                            w_t[:, p_, :, ts(m, P)],
                                            src[:, 2 * p_:2 * p_ + 2,
                                                ts(tb, TB)],
                                            start=(p_ == 0),
                                            stop=(p_ == n_dc // 2 - 1),
                                            perf_mode=DRow)
                                    if on_dve:
                                        nc.vector.tensor_scalar_add(
                                            out[:, g, t, ts(tb, TB)], ps[:],
                                            b_t_[:, m:m + 1])
                                    else:
                                        nc.scalar.activation(
                                            out[:, g, t, ts(tb, TB)], ps[:],
                                            AF.Identity,
                                            bias=b_t_[:, m:m + 1])
                    kq_proj(wk_t, bk_t, KT, T, XN, on_dve=True)
                    kq_proj(wq_t, bq_t, QT, Q, XNQ)

                    NO = 512
                    for to in range(n_tk):
                        for no in range(Dm // NO):
                            ps = ps_pr.tile([P, NO], F32, tag="ps_pr",
                                            name="ps_pr")
                            for p_ in range(n_dc // 2):
                                nc.tensor.matmul(
                                    ps[:],
                                    XN[:, 2 * p_:2 * p_ + 2, ts(to, P)],
                                    wv_t[:, p_, :, ts(no, NO)],
                                    start=(p_ == 0),
                                    stop=(p_ == n_dc // 2 - 1),
                                    perf_mode=DRow)
                            h0 = no * (NO // dh)
                            nc.vector.tensor_tensor(
                                VT[:, to, h0:h0 + NO // dh, 0:dh],
                                ps[:].rearrange("p (h e) -> p h e", e=dh),
                                bv_bc[:, ts(no, NO)]
                                .rearrange("p (h e) -> p h e", e=dh),
                                ALU.add)

                    if dbg:
                        for dc in range(n_dc):
                            nc.sync.dma_start(dbg_d["dXN"][ts(dc, P), :],
                                              XN[:, dc, :])
                        nc.sync.dma_start(dbg_d["dKT"][:, :, :, :], KT[:])
                        nc.sync.dma_start(dbg_d["dQT"][:, :, :, :], QT[:])
                        nc.sync.dma_start(dbg_d["dVT"][:, :, :, :], VT[:])

                # ---------- Phase C/D : attention pipelined with MLP ----------
                with tc.tile_pool(name="p_exps", bufs=3) as p_exps, \
                     tc.tile_pool(name="p_ctm", bufs=2) as p_ctm, \
                     tc.tile_pool(name="p_ct", bufs=1) as p_ct, \
                     tc.tile_pool(name="p_mlp", bufs=1) as p_mlp, \
                     tc.tile_pool(name="p_w1", bufs=4) as p_w1, \
                     tc.tile_pool(name="p_w2", bufs=4) as p_w2, \
                     tc.tile_pool(name="p_st2", bufs=2) as p_st2, \
                     tc.tile_pool(name="p_tm2", bufs=2) as p_tm2, \
                     tc.tile_pool(name="p_out", bufs=2) as p_out, \
                     tc.tile_pool(name="ps_sc", bufs=2, space="PSUM") as ps_sc, \
                     tc.tile_pool(name="ps_cx", bufs=1, space="PSUM") as ps_cx, \
                     tc.tile_pool(name="ps_mx", bufs=2, space="PSUM") as ps_mx, \
                     tc.tile_pool(name="ps_s2", bufs=1, space="PSUM") as ps_s2p:

                    CT = p_ct.tile([P, n_dc, Q], FP8)
                    XN2 = p_mlp.tile([P, n_dc, Q], BF16)
                    Y1 = {}
                    cx_cur = [None]
                    ctm_cur = [None]

                    def attn_unit(qc, h):
                        g, b4 = h // 4, h % 4
                        r0 = 32 * b4
                        qsl = ts(qc, P)
                        if h % 8 == 0:
                            ctm_cur[0] = p_ctm.tile([P, 8, dh], BF16,
                                                    tag="ctm", name="ctm")
                        if h % 4 == 0:
                            cx_cur[0] = ps_cx.tile([P, 4, P], F32,
                                                   tag="ps_cx", name="ps_cx")
                        exps = p_exps.tile([P, n_tk, P], FP8, tag="exps",
                                           name="exps")
                        for half in range(2):
                            pss = ps_sc.tile([P, n_tk // 2, P], F32,
                                             tag="ps_sc", name="ps_sc")
                            for i in range(n_tk // 2):
                                kc = half * (n_tk // 2) + i
                                nc.tensor.matmul(
                                    pss[:, i, :],
                                    KT[r0:r0 + 32, g, :, ts(kc, P)],
                                    QT[r0:r0 + 32, g, :, qsl],
                                    start=True, stop=True, perf_mode=DRow,
                                    tile_position=(r0, 0))
                            nc.scalar.activation(
                                exps[:, half * (n_tk // 2):
                                     (half + 1) * (n_tk // 2), :],
                                pss[:, :, :], AF.Exp, scale=0.125,
                                bias=nb3_t[:, :])
                        ps_c = cx_cur[0]
                        for i in range(n_tk // 2):
                            nc.tensor.matmul(
                                ps_c[:, h % 4, 0:dh + 1],
                                exps[:, 2 * i:2 * i + 2, :],
                                VT[:, 2 * i:2 * i + 2, h, :],
                                start=(i == 0), stop=(i == n_tk // 2 - 1),
                                perf_mode=DRow)
                        if h % 4 == 3:
                            grp = h // 4
                            ctm = ctm_cur[0]
                            rb = p_st2.tile([P, 4, 1], F32, tag="rb",
                                            name="rb")
                            nc.vector.reciprocal(rb[:],
                                                 ps_c[:, :, dh:dh + 1])
                            nc.vector.tensor_tensor(
                                ctm[:, (grp % 2) * 4:(grp % 2) * 4 + 4, :],
                                ps_c[:, :, 0:dh],
                                rb[:].broadcast_to((P, 4, dh)), ALU.mult)
                            if grp % 2 == 1:
                                for j in range(4):
                                    dc = (grp // 2) * 4 + j
                                    tr = ps_mx.tile([P, P], BF16,
                                                    tag="ps_mx", name="tr")
                                    nc.tensor.transpose(
                                        tr[:], ctm[:, 2 * j:2 * j + 2, :],
                                        ident[:])
                                    nc.vector.tensor_copy(
                                        CT[:, dc, ts(qc, P)], tr[:])

                    def emit_attn(qc):
                        for h in range(Hh):
                            attn_unit(qc, h)

                    def emit_wo_ln2(c):
                        tok0, CH = CHUNKS[c]
                        csl = slice(tok0, tok0 + CH)
                        for m in range(n_dc):
                            ps = ps_mx.tile([P, CH], F32, tag="ps_mx",
                                            name="ps_mo")
                            for p_ in range(n_dc // 2):
                                nc.tensor.matmul(
                                    ps[:], wo_t[:, p_, :, ts(m, P)],
                                    CT[:, 2 * p_:2 * p_ + 2, csl],
                                    start=(p_ == 0),
                                    stop=(p_ == n_dc // 2 - 1),
                                    perf_mode=DRow)
                            t1 = p_tm2.tile([P, CH], BF16, tag="t1",
                                            name="t1")
                            nc.vector.tensor_tensor(t1[:], ps[:],
                                                    XQ[:, m, csl], ALU.add)
                            nc.gpsimd.tensor_scalar_add(XQ[:, m, csl], t1[:],
                                                        bo_t[:, m:m + 1])
                        # LN2 on the chunk
                        ps_m = ps_s2p.tile([1, CH], F32, tag="ps_s2",
                                           name="ps_m2")
                        for dc in range(n_dc):
                            nc.tensor.matmul(ps_m[:], ones_h[:],
                                             XQ[:, dc, csl],
                                             start=(dc == 0),
                                             stop=(dc == n_dc - 1))
                        mean = p_st2.tile([1, CH], F32, tag="mean2",
                                          name="mean2")
                        nc.vector.tensor_scalar_mul(mean[:], ps_m[:], inv_d)
                        ps_s = ps_s2p.tile([1, CH], F32, tag="ps_s2",
                                           name="ps_s2")
                        for dc in range(n_dc):
                            sq = p_tm2.tile([P, CH], BF16, tag="sq2",
                                            name="sq2", bufs=1)
                            nc.vector.tensor_tensor(sq[:],
                                                    XQ[:, dc, csl],
                                                    XQ[:, dc, csl], ALU.mult)
                            nc.tensor.matmul(ps_s[:], ones_h[:], sq[:],
                                             start=(dc == 0),
                                             stop=(dc == n_dc - 1))
                        ex2 = p_st2.tile([1, CH], F32, tag="ex22",
                                         name="ex22")
                        nc.vector.tensor_scalar_mul(ex2[:], ps_s[:], inv_d)
                        var = p_st2.tile([1, CH], F32, tag="var2",
                                         name="var2")
                        nc.vector.tensor_tensor(var[:], mean[:], mean[:],
                                                ALU.mult)
                        nc.vector.tensor_tensor(var[:], ex2[:], var[:],
                                                ALU.subtract)
                        std = p_st2.tile([1, CH], F32, tag="std2",
                                         name="std2")
                        nc.scalar.activation(std[:], var[:], AF.Sqrt,
                                             bias=eps_t[:, :])
                        rstd = p_st2.tile([1, CH], F32, tag="rstd2",
                                          name="rstd2")
                        nc.vector.reciprocal(rstd[:], std[:])
                        mean_h = p_st2.tile([1, CH], BF16, tag="mean2h",
                                            name="mean2h")
                        nc.vector.tensor_copy(mean_h[:], mean[:])
                        rstd_h = p_st2.tile([1, CH], BF16, tag="rstd2h",
                                            name="rstd2h")
                        nc.vector.tensor_copy(rstd_h[:], rstd[:])
                        mbc = p_tm2.tile([P, CH], BF16, tag="mbc2",
                                         name="mbc2", bufs=1)
                        nc.gpsimd.partition_broadcast(mbc[:], mean_h[:])
                        rbc = p_tm2.tile([P, CH], BF16, tag="rbc2",
                                         name="rbc2", bufs=1)
                        nc.gpsimd.partition_broadcast(rbc[:], rstd_h[:])
                        for dc in range(n_dc):
                            t0 = p_tm2.tile([P, CH], BF16, tag="t02",
                                            name="t02")
                            nc.vector.tensor_tensor(t0[:], XQ[:, dc, csl],
                                                    mbc[:], ALU.subtract)
                            nc.vector.tensor_tensor(t0[:], t0[:], rbc[:],
                                                    ALU.mult)
                            nc.vector.tensor_scalar(
                                XN2[:, dc, csl], t0[:], g2_t[:, dc:dc + 1],
                                be2_t[:, dc:dc + 1], ALU.mult, ALU.add)

                    def emit_fc1(c, mo0, mo1):
                        tok0, CH = CHUNKS[c]
                        csl = slice(tok0, tok0 + CH)
                        if mo0 == 0:
                            Y1[c] = p_mlp.tile([P, n_mo, CHUNKS[c][1]], BF16,
                                               tag="y1", name="y1", bufs=1,
                                               padded_shape=[P, n_mo, 384])
                        for mo in range(mo0, mo1):
                            wt = p_w1.tile([P, n_dc, P], BF16, tag="w1",
                                           name="w1")
                            nc.sync.dma_start(
                                wt[:],
                                w1_d[:, ts(mo, P)]
                                .rearrange("(c p) m -> p c m", p=P))
                            ps = ps_mx.tile([P, CH], F32, tag="ps_mx",
                                            name="ps_f1")
                            for dc in range(n_dc):
                                nc.tensor.matmul(ps[:], wt[:, dc, :],
                                                 XN2[:, dc, csl],
                                                 start=(dc == 0),
                                                 stop=(dc == n_dc - 1))
                            nc.vector.tensor_scalar_add(Y1[c][:, mo, :],
                                                        ps[:],
                                                        b1_t[:, mo:mo + 1])

                    def emit_gelu(c):
                        for g8 in range(0, n_mo, 16):
                            nc.scalar.activation(Y1[c][:, g8:g8 + 16, :],
                                                 Y1[c][:, g8:g8 + 16, :],
                                                 AF.Gelu)

                    def emit_fc2(c):
                        tok0, CH = CHUNKS[c]
                        csl = slice(tok0, tok0 + CH)
                        for m2 in range(n_dc):
                            wt = p_w2.tile([P, n_mo // 2, P], BF16, tag="w2",
                                           name="w2")
                            nc.sync.dma_start(
                                wt[:],
                                w2_d[0:Mlp // 2, ts(m2, P)]
                                .rearrange("(c p) m -> p c m", p=P))
                            wt2 = p_w2.tile([P, n_mo // 2, P], BF16, tag="w2",
                                            name="w2b")
                            nc.sync.dma_start(
                                wt2[:],
                                w2_d[Mlp // 2:Mlp, ts(m2, P)]
                                .rearrange("(c p) m -> p c m", p=P))
                            ps = ps_mx.tile([P, CH], F32, tag="ps_mx",
                                            name="ps_f2")
                            for kc in range(n_mo):
                                w_ = wt if kc < n_mo // 2 else wt2
                                nc.tensor.matmul(ps[:],
                                                 w_[:, kc % (n_mo // 2), :],
                                                 Y1[c][:, kc, :],
                                                 start=(kc == 0),
                                                 stop=(kc == n_mo - 1))
                            ot = p_out.tile([P, CH], F32, tag="out",
                                            name="out")
                            nc.vector.tensor_tensor(ot[:], ps[:],
                                                    XQ[:, m2, csl], ALU.add)
                            nc.gpsimd.tensor_scalar_add(ot[:], ot[:],
                                                        b2_t[:, m2:m2 + 1])
                            nc.sync.dma_start(yT_d[ts(m2, P), csl], ot[:])

                    # 3-chunk software pipeline: MLP of chunk c emitted
                    # under the attention of later query chunks.
                    emit_attn(0)
                    emit_attn(1)
                    emit_attn(2)
                    emit_wo_ln2(0)
                    emit_attn(3)
                    emit_fc1(0, 0, 32)
                    emit_attn(4)
                    emit_gelu(0)
                    emit_fc2(0)
                    emit_wo_ln2(1)
                    emit_attn(5)
                    emit_fc1(1, 0, 16)
                    emit_attn(6)
                    emit_fc1(1, 16, 32)
                    emit_attn(7)
                    emit_gelu(1)
                    emit_fc2(1)
                    emit_wo_ln2(2)
                    emit_fc1(2, 0, 32)
                    emit_gelu(2)
                    emit_fc2(2)
                    if dbg:
                        for dc in range(n_dc):
                            nc.sync.dma_start(dbg_d["dCT"][ts(dc, P), :],
                                              CT[:, dc, :])
                            nc.sync.dma_start(dbg_d["dH2"][ts(dc, P), :],
                                              XQ[:, dc, :])
                            nc.sync.dma_start(dbg_d["dXN2"][ts(dc, P), :],
                                              XN2[:, dc, :])
    nc.compile()
    return nc


_NC_CACHE = {}


def _get_nc(T, Q, Dm, Hh, Mlp, n_cores):
    key = (T, Q, Dm, Hh, Mlp, n_cores)
    if key not in _NC_CACHE:
        _NC_CACHE[key] = build_bass(T, Q, Dm, Hh, Mlp, n_cores)
    return _NC_CACHE[key]


def _perm_cols(Dm):
    """Column permutation for the K/Q DoubleRow band layout.
    Chunk m=2g+t, partition p=32*b4+r  ->  original feature
    (4g+b4)*64 + t*32 + r."""
    perm = np.empty(Dm, np.int64)
    for m in range(Dm // P):
        g, t = m // 2, m % 2
        for p in range(P):
            b4, r = p // 32, p % 32
            perm[m * P + p] = (4 * g + b4) * 64 + t * 32 + r
    return perm


def _dr_weight(w, scale=1.0):
    """[D, N] -> (128, D/256, 2, N) fp8: [r, p, t, c] = w[(2p+t)*128+r, c]."""
    Dm = w.shape[0]
    f8 = ml_dtypes.float8_e4m3
    return np.ascontiguousarray(
        (w.reshape(Dm // 256, 2, P, w.shape[1]) * scale)
        .transpose(2, 0, 1, 3)).astype(f8)


def make_in_maps(inputs, n_cores):
    x = np.asarray(inputs["x"], np.float32)
    Bq, Sq, Dq = x.shape
    Qtok = Sq * Bq // n_cores
    bf = ml_dtypes.bfloat16
    perm = _perm_cols(Dq)
    wk = np.asarray(inputs["Wk"], np.float32)[:, perm]
    wq = np.asarray(inputs["Wq"], np.float32)[:, perm]
    shared = {
        "g1": np.asarray(inputs["ln1_g"], np.float32),
        "be1": np.asarray(inputs["ln1_b"], np.float32),
        "g2": np.asarray(inputs["ln2_g"], np.float32),
        "be2": np.asarray(inputs["ln2_b"], np.float32),
        "wk8": _dr_weight(wk),
        "wq8": _dr_weight(wq),
        "wv8": _dr_weight(np.asarray(inputs["Wv"], np.float32)),
        "wo8": _dr_weight(np.asarray(inputs["Wo"], np.float32)),
        "w1h": np.asarray(inputs["W1"], np.float32).astype(bf),
        "w2h": np.asarray(inputs["W2"], np.float32).astype(bf),
        "bkp": np.asarray(inputs["bk"], np.float32)[perm],
        "bqp": np.asarray(inputs["bq"], np.float32)[perm],
        "bv": np.asarray(inputs["bv"], np.float32),
        "bo": np.asarray(inputs["bo"], np.float32),
        "b1": np.asarray(inputs["b1"], np.float32),
        "b2": np.asarray(inputs["b2"], np.float32),
        "ones16": np.ones((P, 1), bf),
        "ident16": np.eye(P, dtype=bf),
    }
    in_maps = []
    for c in range(n_cores):
        b = c // (n_cores // Bq)
        qoff = (c % (n_cores // Bq)) * Qtok
        m = dict(shared)
        xrot = np.concatenate([x[b, qoff:], x[b, :qoff]], axis=0)
        m["xT"] = np.ascontiguousarray(xrot.T)
        in_maps.append(m)
    return in_maps, Qtok


def kernel(**inputs):
    x = np.asarray(inputs["x"], np.float32)
    Bq, Sq, Dq = x.shape
    in_maps, Qtok = make_in_maps(inputs, N_CORES)
    nc = _get_nc(Sq, Qtok, Dq, H, MLP, N_CORES)
    res = run_bass_kernel_spmd(nc, in_maps, core_ids=list(range(N_CORES)))
    out = np.empty((Bq, Sq, Dq), np.float32)
    per_b = N_CORES // Bq
    for c in range(N_CORES):
        b = c // per_b
        qoff = (c % per_b) * Qtok
        out[b, qoff:qoff + Qtok, :] = res.results[c]["yT"].T
    return out


# revision 29
# speedup vs baseline: 1.0866x; 1.0257x over previous
"""Trainium2 Bass kernel for a dense transformer block (LN1 -> MHA -> LN2 -> MLP).

Sharding: 8 cores = (batch b in 0..3) x (sequence half in 0..1). Each core
computes the block output for its 1024 query tokens; K/V are computed for the
batch's full 2048 tokens on each core, so there is zero cross-core traffic.

v2 design (cost-model-driven):
- fp8e4 DoubleRow matmuls (0.5 cyc/row, 256-deep contraction) for QKV/Wo
  projections, the scores (32-partition head-band layout via host-permuted
  Wk/Wq columns), and probs @ V.
- ctx computed in [q, dh] orientation with a fused ones-column on V so the
  softmax denominator falls out of the same accumulation chain, then
  PE-transposed (bf16) back to feature-major for Wo.
- exp carries a -3 bias (cancels in normalization) so fp8e4 probs can't
  overflow; ctx is scaled x16 before the fp8 cast and Wo pre-scaled /16.
- MLP stays bf16 for accuracy. 2-stage software pipeline: attention of the
  second 512 queries is emitted interleaved with the MLP of the first 512.
- Elementwise work split across DVE / Act / GpSimd; Gelu/Sqrt (table-based)
  batched so act-table reloads stay rare (Copy/Square/Identity are in every
  table set and interleave freely with Exp).
"""

import sys

if '/opt/trn_rl_repo' not in sys.path:
    sys.path.insert(0, '/opt/trn_rl_repo')

import numpy as np
import ml_dtypes

import concourse.tile as tile
import concourse.mybir as mybir
from concourse import bacc
from concourse.bass import ts
from concourse.bass_utils import run_bass_kernel_spmd

P = 128
F32 = mybir.dt.float32
BF16 = mybir.dt.bfloat16
FP8 = mybir.dt.float8e4
AF = mybir.ActivationFunctionType
ALU = mybir.AluOpType
DRow = mybir.MatmulPerfMode.DoubleRow
EPS = 1e-6
EXP_BIAS = -3.0

B, S, D, H, MLP = 4, 2048, 1024, 16, 4096
N_CORES = 8


def build_bass(T, Q, Dm, Hh, Mlp, n_cores, dbg=False):
    dh = Dm // Hh
    assert dh == 64
    n_dc = Dm // P          # 8 feature chunks
    n_tk = T // P           # 16 key chunks
    n_qc = Q // P           # 8 query chunks
    n_mo = Mlp // P         # 32 mlp chunks
    n_g = Hh // 4           # 4 head quads
    TB = 512
    CHUNKS = [(0, 256), (256, 384), (640, 384)]   # mlp pipeline chunks
    inv_d = 1.0 / Dm

    nc = bacc.Bacc("TRN2", target_bir_lowering=False, debug=False,
                   enable_asserts=False, num_devices=n_cores)

    def din(name, shape, dt):
        return nc.dram_tensor(name, shape, dt, kind="ExternalInput").ap()

    xT_d = din("xT", (Dm, T), F32)
    g1_d, be1_d = din("g1", (Dm,), F32), din("be1", (Dm,), F32)
    g2_d, be2_d = din("g2", (Dm,), F32), din("be2", (Dm,), F32)
    wk8_d = din("wk8", (P, n_dc // 2, 2, Dm), FP8)
    wq8_d = din("wq8", (P, n_dc // 2, 2, Dm), FP8)
    wv8_d = din("wv8", (P, n_dc // 2, 2, Dm), FP8)
    wo8_d = din("wo8", (P, n_dc // 2, 2, Dm), FP8)
    w1_d = din("w1h", (Dm, Mlp), BF16)
    w2_d = din("w2h", (Mlp, Dm), BF16)
    bq_d, bk_d = din("bqp", (Dm,), F32), din("bkp", (Dm,), F32)
    bv_d, bo_d = din("bv", (Dm,), F32), din("bo", (Dm,), F32)
    b1_d, b2_d = din("b1", (Mlp,), F32), din("b2", (Dm,), F32)
    ones_d = din("ones16", (P, 1), BF16)
    ident_d = din("ident16", (P, P), BF16)
    yT_d = nc.dram_tensor("yT", (Dm, Q), F32, kind="ExternalOutput").ap()
    dbg_d = {}
    if dbg:
        for nm, shape, dt in [("dXN", (Dm, T), FP8), ("dKT", (P, n_g, 2, T), FP8),
                              ("dQT", (P, n_g, 2, Q), FP8),
                              ("dVT", (P, n_tk, Hh, dh + 1), FP8),
                              ("dCT", (Dm, Q), FP8), ("dH2", (Dm, Q), BF16),
                              ("dXN2", (Dm, Q), BF16)]:
            dbg_d[nm] = nc.dram_tensor(nm, shape, dt, kind="ExternalOutput").ap()

    with tile.TileContext(nc) as tc:
        with tc.tile_pool(name="const", bufs=1) as constp:
            ones_h = constp.tile([P, 1], BF16)
            nc.sync.dma_start(ones_h[:], ones_d[:, :])
            ident = constp.tile([P, P], BF16)
            nc.sync.dma_start(ident[:], ident_d[:, :])
            eps_t = constp.tile([1, 1], F32)
            nc.vector.memset(eps_t[:], EPS)
            nb3_t = constp.tile([P, 1], F32)
            nc.vector.memset(nb3_t[:], EXP_BIAS)

            def vec_tile(src, n, nm):
                t = constp.tile([P, n], F32, tag=nm, name=nm)
                nc.sync.dma_start(t[:], src.rearrange("(c p) -> p c", p=P))
                return t

            g1_t, be1_t = vec_tile(g1_d, n_dc, "g1"), vec_tile(be1_d, n_dc, "be1")
            g2_t, be2_t = vec_tile(g2_d, n_dc, "g2"), vec_tile(be2_d, n_dc, "be2")
            bq_t, bk_t = vec_tile(bq_d, n_dc, "bq"), vec_tile(bk_d, n_dc, "bk")
            bo_t, b2_t = vec_tile(bo_d, n_dc, "bo"), vec_tile(b2_d, n_dc, "b2")
            b1_t = vec_tile(b1_d, n_mo, "b1")
            bv_row = constp.tile([1, Dm], F32)
            nc.sync.dma_start(bv_row[:, :], bv_d[None, :])
            bv_bc = constp.tile([P, Dm], F32)
            nc.gpsimd.partition_broadcast(bv_bc[:], bv_row[:])

            wo_t = constp.tile([P, n_dc // 2, 2, Dm], FP8, name="wo")
            nc.sync.dma_start(wo_t[:], wo8_d[:, :, :, :])

            with tc.tile_pool(name="p_res", bufs=1) as p_res, \
                 tc.tile_pool(name="p_kv", bufs=1) as p_kv:
                XRAW = p_res.tile([P, n_dc, T], BF16)     # raw x (q-first)
                XQ = XRAW[:, :, 0:Q]                      # raw xq, then h2
                KT = p_kv.tile([P, n_g, 2, T], FP8)       # band layout
                QT = p_kv.tile([P, n_g, 2, Q], FP8)
                VT = p_kv.tile([P, n_tk, Hh, dh + 1], FP8)
                nc.gpsimd.memset(VT[:, :, :, dh:dh + 1], 1.0)

                # ---------- Phase A/B : LN1 + QKV projections ----------
                with tc.tile_pool(name="p_act", bufs=1) as p_act, \
                     tc.tile_pool(name="p_tmp", bufs=6) as p_tmp, \
                     tc.tile_pool(name="p_st", bufs=1) as p_st, \
                     tc.tile_pool(name="ps_st", bufs=2, space="PSUM") as ps_st, \
                     tc.tile_pool(name="ps_pr", bufs=6, space="PSUM") as ps_pr:

                    XN = p_act.tile([P, n_dc, T], FP8)
                    XNQ = XN[:, :, 0:Q]

                    def ln_block(n_tok, xdram, xraw, g_t, b_t, out_ap,
                                 preloaded):
                        for tb in range(n_tok // TB):
                            sl = ts(tb, TB)
                            if not preloaded:
                                for dc in range(n_dc):
                                    xc = p_tmp.tile([P, TB], F32, tag="xc",
                                                    name="xc")
                                    nc.sync.dma_start(xc[:],
                                                      xdram[ts(dc, P), sl])
                                    nc.gpsimd.tensor_copy(xraw[:, dc, sl],
                                                          xc[:])
                            ps_m = ps_st.tile([1, TB], F32, tag="ps_st",
                                              name="ps_m")
                            for dc in range(n_dc):
                                nc.tensor.matmul(ps_m[:], ones_h[:],
                                                 xraw[:, dc, sl],
                                                 start=(dc == 0),
                                                 stop=(dc == n_dc - 1))
                            sq = p_tmp.tile([P, n_dc, TB], BF16, tag="sq",
                                            name="sq", bufs=1)
                            for dc in range(n_dc):
                                nc.vector.tensor_tensor(
                                    sq[:, dc, :], xraw[:, dc, sl],
                                    xraw[:, dc, sl], ALU.mult)
                            ps_s = ps_st.tile([1, TB], F32, tag="ps_st",
                                              name="ps_s")
                            for dc in range(n_dc):
                                nc.tensor.matmul(ps_s[:], ones_h[:],
                                                 sq[:, dc, :],
                                                 start=(dc == 0),
                                                 stop=(dc == n_dc - 1))
                            mean = p_st.tile([1, TB], F32, tag="mean",
                                             name="mean")
                            nc.vector.tensor_scalar_mul(mean[:], ps_m[:],
                                                        inv_d)
                            ex2 = p_st.tile([1, TB], F32, tag="ex2",
                                            name="ex2")
                            nc.vector.tensor_scalar_mul(ex2[:], ps_s[:],
                                                        inv_d)
                            var = p_st.tile([1, TB], F32, tag="var",
                                            name="var")
                            nc.vector.tensor_tensor(var[:], mean[:], mean[:],
                                                    ALU.mult)
                            nc.vector.tensor_tensor(var[:], ex2[:], var[:],
                                                    ALU.subtract)
                            std = p_st.tile([1, TB], F32, tag="std",
                                            name="std")
                            nc.scalar.activation(std[:], var[:], AF.Sqrt,
                                                 bias=eps_t[:, :])
                            rstd = p_st.tile([1, TB], F32, tag="rstd",
                                             name="rstd")
                            nc.vector.reciprocal(rstd[:], std[:])
                            mean_h = p_st.tile([1, TB], BF16, tag="mean_h",
                                               name="mean_h", bufs=2)
                            nc.vector.tensor_copy(mean_h[:], mean[:])
                            rstd_h = p_st.tile([1, TB], BF16, tag="rstd_h",
                                               name="rstd_h", bufs=2)
                            nc.vector.tensor_copy(rstd_h[:], rstd[:])
                            mbc = p_tmp.tile([P, TB], BF16, tag="mbc",
                                             name="mbc", bufs=2)
                            nc.gpsimd.partition_broadcast(mbc[:], mean_h[:])
                            rbc = p_tmp.tile([P, TB], BF16, tag="rbc",
                                             name="rbc", bufs=2)
                            nc.gpsimd.partition_broadcast(rbc[:], rstd_h[:])
                            for dc in range(n_dc):
                                t0 = p_tmp.tile([P, TB], BF16, tag="t0",
                                                name="t0")
                                nc.vector.tensor_tensor(
                                    t0[:], xraw[:, dc, sl], mbc[:],
                                    ALU.subtract)
                                nc.vector.tensor_tensor(t0[:], t0[:], rbc[:],
                                                        ALU.mult)
                                nc.scalar.activation(
                                    out_ap[:, dc, sl], t0[:], AF.Identity,
                                    bias=b_t[:, dc:dc + 1],
                                    scale=g_t[:, dc:dc + 1])

                    ln_block(T, xT_d, XRAW, g1_t, be1_t, XN, False)
                    wk_t = p_act.tile([P, n_dc // 2, 2, Dm], FP8, name="wk")
                    nc.sync.dma_start(wk_t[:], wk8_d[:, :, :, :])
                    wq_t = p_act.tile([P, n_dc // 2, 2, Dm], FP8, name="wq")
                    nc.sync.dma_start(wq_t[:], wq8_d[:, :, :, :])
                    wv_t = p_act.tile([P, n_dc // 2, 2, Dm], FP8, name="wv")
                    nc.sync.dma_start(wv_t[:], wv8_d[:, :, :, :])

                    def kq_proj(w_t, b_t_, out, n_tok, src, split=False):
                        for g in range(n_g):
                            for t in range(2):
                                m = 2 * g + t
                                for tb in range(n_tok // TB):
                                    ps = ps_pr.tile([P, TB], F32, tag="ps_pr",
                                                    name="ps_pr")
                                    for p_ in range(n_dc // 2):
                                        nc.tensor.matmul(
                                            ps[:],
                                            w_t[:, p_, :, ts(m, P)],
                                            src[:, 2 * p_:2 * p_ + 2,
                                                ts(tb, TB)],
                                            start=(p_ == 0),
                                            stop=(p_ == n_dc // 2 - 1),
                                            perf_mode=DRow)
                                    if split and tb % 2 == 0:
                                        nc.vector.tensor_scalar_add(
                                            out[:, g, t, ts(tb, TB)], ps[:],
                                            b_t_[:, m:m + 1])
                                    else:
                                        nc.scalar.activation(
                                            out[:, g, t, ts(tb, TB)], ps[:],
                                            AF.Identity,
                                            bias=b_t_[:, m:m + 1])
                    kq_proj(wk_t, bk_t, KT, T, XN, split=True)
                    kq_proj(wq_t, bq_t, QT, Q, XNQ)

                    NO = 512
                    for to in range(n_tk):
                        for no in range(Dm // NO):
                            ps = ps_pr.tile([P, NO], F32, tag="ps_pr",
                                            name="ps_pr")
                            for p_ in range(n_dc // 2):
                                nc.tensor.matmul(
                                    ps[:],
                                    XN[:, 2 * p_:2 * p_ + 2, ts(to, P)],
                                    wv_t[:, p_, :, ts(no, NO)],
                                    start=(p_ == 0),
                                    stop=(p_ == n_dc // 2 - 1),
                                    perf_mode=DRow)
                            h0 = no * (NO // dh)
                            nc.vector.tensor_tensor(
                                VT[:, to, h0:h0 + NO // dh, 0:dh],
                                ps[:].rearrange("p (h e) -> p h e", e=dh),
                                bv_bc[:, ts(no, NO)]
                                .rearrange("p (h e) -> p h e", e=dh),
                                ALU.add)

                    if dbg:
                        for dc in range(n_dc):
                            nc.sync.dma_start(dbg_d["dXN"][ts(dc, P), :],
                                              XN[:, dc, :])
                        nc.sync.dma_start(dbg_d["dKT"][:, :, :, :], KT[:])
                        nc.sync.dma_start(dbg_d["dQT"][:, :, :, :], QT[:])
                        nc.sync.dma_start(dbg_d["dVT"][:, :, :, :], VT[:])

                # ---------- Phase C/D : attention pipelined with MLP ----------
                with tc.tile_pool(name="p_exps", bufs=3) as p_exps, \
                     tc.tile_pool(name="p_ctm", bufs=2) as p_ctm, \
                     tc.tile_pool(name="p_ct", bufs=1) as p_ct, \
                     tc.tile_pool(name="p_mlp", bufs=1) as p_mlp, \
                     tc.tile_pool(name="p_w1", bufs=4) as p_w1, \
                     tc.tile_pool(name="p_w2", bufs=4) as p_w2, \
                     tc.tile_pool(name="p_st2", bufs=2) as p_st2, \
                     tc.tile_pool(name="p_tm2", bufs=2) as p_tm2, \
                     tc.tile_pool(name="p_out", bufs=2) as p_out, \
                     tc.tile_pool(name="ps_sc", bufs=2, space="PSUM") as ps_sc, \
                     tc.tile_pool(name="ps_cx", bufs=1, space="PSUM") as ps_cx, \
                     tc.tile_pool(name="ps_mx", bufs=2, space="PSUM") as ps_mx, \
                     tc.tile_pool(name="ps_s2", bufs=1, space="PSUM") as ps_s2p:

                    CT = p_ct.tile([P, n_dc, Q], FP8)
                    XN2 = p_mlp.tile([P, n_dc, Q], BF16)
                    Y1 = {}
                    cx_cur = [None]
                    ctm_cur = [None]

                    def attn_unit(qc, h):
                        g, b4 = h // 4, h % 4
                        r0 = 32 * b4
                        qsl = ts(qc, P)
                        if h % 8 == 0:
                            ctm_cur[0] = p_ctm.tile([P, 8, dh], BF16,
                                                    tag="ctm", name="ctm")
                        if h % 4 == 0:
                            cx_cur[0] = ps_cx.tile([P, 4, P], F32,
                                                   tag="ps_cx", name="ps_cx")
                        exps = p_exps.tile([P, n_tk, P], FP8, tag="exps",
                                           name="exps")
                        for half in range(2):
                            pss = ps_sc.tile([P, n_tk // 2, P], F32,
                                             tag="ps_sc", name="ps_sc")
                            for i in range(n_tk // 2):
                                kc = half * (n_tk // 2) + i
                                nc.tensor.matmul(
                                    pss[:, i, :],
                                    KT[r0:r0 + 32, g, :, ts(kc, P)],
                                    QT[r0:r0 + 32, g, :, qsl],
                                    start=True, stop=True, perf_mode=DRow,
                                    tile_position=(r0, 0))
                            nc.scalar.activation(
                                exps[:, half * (n_tk // 2):
                                     (half + 1) * (n_tk // 2), :],
                                pss[:, :, :], AF.Exp, scale=0.125,
                                bias=nb3_t[:, :])
                        ps_c = cx_cur[0]
                        for i in range(n_tk // 2):
                            nc.tensor.matmul(
                                ps_c[:, h % 4, 0:dh + 1],
                                exps[:, 2 * i:2 * i + 2, :],
                                VT[:, 2 * i:2 * i + 2, h, :],
                                start=(i == 0), stop=(i == n_tk // 2 - 1),
                                perf_mode=DRow)
                        if h % 4 == 3:
                            grp = h // 4
                            ctm = ctm_cur[0]
                            rb = p_st2.tile([P, 4, 1], F32, tag="rb",
                                            name="rb")
                            nc.vector.reciprocal(rb[:],
                                                 ps_c[:, :, dh:dh + 1])
                            nc.vector.tensor_tensor(
                                ctm[:, (grp % 2) * 4:(grp % 2) * 4 + 4, :],
                                ps_c[:, :, 0:dh],
                                rb[:].broadcast_to((P, 4, dh)), ALU.mult)
                            if grp % 2 == 1:
                                for j in range(4):
                                    dc = (grp // 2) * 4 + j
                                    tr = ps_mx.tile([P, P], BF16,
                                                    tag="ps_mx", name="tr")
                                    nc.tensor.transpose(
                                        tr[:], ctm[:, 2 * j:2 * j + 2, :],
                                        ident[:])
                                    nc.vector.tensor_copy(
                                        CT[:, dc, ts(qc, P)], tr[:])

                    def emit_attn(qc):
                        for h in range(Hh):
                            attn_unit(qc, h)

                    def emit_wo_ln2(c):
                        tok0, CH = CHUNKS[c]
                        csl = slice(tok0, tok0 + CH)
                        for m in range(n_dc):
                            ps = ps_mx.tile([P, CH], F32, tag="ps_mx",
                                            name="ps_mo")
                            for p_ in range(n_dc // 2):
                                nc.tensor.matmul(
                                    ps[:], wo_t[:, p_, :, ts(m, P)],
                                    CT[:, 2 * p_:2 * p_ + 2, csl],
                                    start=(p_ == 0),
                                    stop=(p_ == n_dc // 2 - 1),
                                    perf_mode=DRow)
                            t1 = p_tm2.tile([P, CH], BF16, tag="t1",
                                            name="t1")
                            nc.vector.tensor_tensor(t1[:], ps[:],
                                                    XQ[:, m, csl], ALU.add)
                            nc.gpsimd.tensor_scalar_add(XQ[:, m, csl], t1[:],
                                                        bo_t[:, m:m + 1])
                        # LN2 on the chunk
                        ps_m = ps_s2p.tile([1, CH], F32, tag="ps_s2",
                                           name="ps_m2")
                        for dc in range(n_dc):
                            nc.tensor.matmul(ps_m[:], ones_h[:],
                                             XQ[:, dc, csl],
                                             start=(dc == 0),
                                             stop=(dc == n_dc - 1))
                        mean = p_st2.tile([1, CH], F32, tag="mean2",
                                          name="mean2")
                        nc.vector.tensor_scalar_mul(mean[:], ps_m[:], inv_d)
                        ps_s = ps_s2p.tile([1, CH], F32, tag="ps_s2",
                                           name="ps_s2")
                        for dc in range(n_dc):
                            sq = p_tm2.tile([P, CH], BF16, tag="sq2",
                                            name="sq2", bufs=1)
                            nc.vector.tensor_tensor(sq[:],
                                                    XQ[:, dc, csl],
                                                    XQ[:, dc, csl], ALU.mult)
                            nc.tensor.matmul(ps_s[:], ones_h[:], sq[:],
                                             start=(dc == 0),
                                             stop=(dc == n_dc - 1))
                        ex2 = p_st2.tile([1, CH], F32, tag="ex22",
                                         name="ex22")
                        nc.vector.tensor_scalar_mul(ex2[:], ps_s[:], inv_d)
                        var = p_st2.tile([1, CH], F32, tag="var2",
                                         name="var2")
                        nc.vector.tensor_tensor(var[:], mean[:], mean[:],
                                                ALU.mult)
                        nc.vector.tensor_tensor(var[:], ex2[:], var[:],
                                                ALU.subtract)
                        std = p_st2.tile([1, CH], F32, tag="std2",
                                         name="std2")
                        nc.scalar.activation(std[:], var[:], AF.Sqrt,
                                             bias=eps_t[:, :])
                        rstd = p_st2.tile([1, CH], F32, tag="rstd2",
                                          name="rstd2")
                        nc.vector.reciprocal(rstd[:], std[:])
                        mean_h = p_st2.tile([1, CH], BF16, tag="mean2h",
                                            name="mean2h")
                        nc.vector.tensor_copy(mean_h[:], mean[:])
                        rstd_h = p_st2.tile([1, CH], BF16, tag="rstd2h",
                                            name="rstd2h")
                        nc.vector.tensor_copy(rstd_h[:], rstd[:])
                        mbc = p_tm2.tile([P, CH], BF16, tag="mbc2",
                                         name="mbc2", bufs=1)
                        nc.gpsimd.partition_broadcast(mbc[:], mean_h[:])
                        rbc = p_tm2.tile([P, CH], BF16, tag="rbc2",
                                         name="rbc2", bufs=1)
                        nc.gpsimd.partition_broadcast(rbc[:], rstd_h[:])
                        for dc in range(n_dc):
                            t0 = p_tm2.tile([P, CH], BF16, tag="t02",
                                            name="t02")
                            nc.vector.tensor_tensor(t0[:], XQ[:, dc, csl],
                                                    mbc[:], ALU.subtract)
                            nc.vector.tensor_tensor(t0[:], t0[:], rbc[:],
                                                    ALU.mult)
                            nc.vector.tensor_scalar(
                                XN2[:, dc, csl], t0[:], g2_t[:, dc:dc + 1],
                                be2_t[:, dc:dc + 1], ALU.mult, ALU.add)

                    def emit_fc1(c, mo0, mo1):
                        tok0, CH = CHUNKS[c]
                        csl = slice(tok0, tok0 + CH)
                        if mo0 == 0:
                            Y1[c] = p_mlp.tile([P, n_mo, CHUNKS[c][1]], BF16,
                                               tag="y1", name="y1", bufs=1,
                                               padded_shape=[P, n_mo, 384])
                        for mo in range(mo0, mo1):
                            wt = p_w1.tile([P, n_dc, P], BF16, tag="w1",
                                           name="w1")
                            nc.sync.dma_start(
                                wt[:],
                                w1_d[:, ts(mo, P)]
                                .rearrange("(c p) m -> p c m", p=P))
                            ps = ps_mx.tile([P, CH], F32, tag="ps_mx",
                                            name="ps_f1")
                            for dc in range(n_dc):
                                nc.tensor.matmul(ps[:], wt[:, dc, :],
                                                 XN2[:, dc, csl],
                                                 start=(dc == 0),
                                                 stop=(dc == n_dc - 1))
                            nc.vector.tensor_scalar_add(Y1[c][:, mo, :],
                                                        ps[:],
                                                        b1_t[:, mo:mo + 1])

                    def emit_gelu(c):
                        for g8 in range(0, n_mo, 16):
                            nc.scalar.activation(Y1[c][:, g8:g8 + 16, :],
                                                 Y1[c][:, g8:g8 + 16, :],
                                                 AF.Gelu)

                    def emit_fc2(c):
                        tok0, CH = CHUNKS[c]
                        csl = slice(tok0, tok0 + CH)
                        for m2 in range(n_dc):
                            wt = p_w2.tile([P, n_mo // 2, P], BF16, tag="w2",
                                           name="w2")
                            nc.sync.dma_start(
                                wt[:],
                                w2_d[0:Mlp // 2, ts(m2, P)]
                                .rearrange("(c p) m -> p c m", p=P))
                            wt2 = p_w2.tile([P, n_mo // 2, P], BF16, tag="w2",
                                            name="w2b")
                            nc.sync.dma_start(
                                wt2[:],
                                w2_d[Mlp // 2:Mlp, ts(m2, P)]
                                .rearrange("(c p) m -> p c m", p=P))
                            ps = ps_mx.tile([P, CH], F32, tag="ps_mx",
                                            name="ps_f2")
                            for kc in range(n_mo):
                                w_ = wt if kc < n_mo // 2 else wt2
                                nc.tensor.matmul(ps[:],
                                                 w_[:, kc % (n_mo // 2), :],
                                                 Y1[c][:, kc, :],
                                                 start=(kc == 0),
                                                 stop=(kc == n_mo - 1))
                            ot = p_out.tile([P, CH], F32, tag="out",
                                            name="out")
                            nc.vector.tensor_tensor(ot[:], ps[:],
                                                    XQ[:, m2, csl], ALU.add)
                            nc.gpsimd.tensor_scalar_add(ot[:], ot[:],
                                                        b2_t[:, m2:m2 + 1])
                            nc.sync.dma_start(yT_d[ts(m2, P), csl], ot[:])

                    # 3-chunk software pipeline: MLP of chunk c emitted
                    # under the attention of later query chunks.
                    emit_attn(0)
                    emit_attn(1)
                    emit_wo_ln2(0)
                    emit_attn(2)
                    emit_fc1(0, 0, 32)
                    emit_attn(3)
                    emit_gelu(0)
                    emit_fc2(0)
                    emit_attn(4)
                    emit_wo_ln2(1)
                    emit_attn(5)
                    emit_fc1(1, 0, 16)
                    emit_attn(6)
                    emit_fc1(1, 16, 32)
                    emit_attn(7)
                    emit_gelu(1)
                    emit_fc2(1)
                    emit_wo_ln2(2)
                    emit_fc1(2, 0, 32)
                    emit_gelu(2)
                    emit_fc2(2)
                    if dbg:
                        for dc in range(n_dc):
                            nc.sync.dma_start(dbg_d["dCT"][ts(dc, P), :],
                                              CT[:, dc, :])
                            nc.sync.dma_start(dbg_d["dH2"][ts(dc, P), :],
                                              XQ[:, dc, :])
                            nc.sync.dma_start(dbg_d["dXN2"][ts(dc, P), :],
                                              XN2[:, dc, :])
    nc.compile()
    return nc


_NC_CACHE = {}


def _get_nc(T, Q, Dm, Hh, Mlp, n_cores):
    key = (T, Q, Dm, Hh, Mlp, n_cores)
    if key not in _NC_CACHE:
        _NC_CACHE[key] = build_bass(T, Q, Dm, Hh, Mlp, n_cores)
    return _NC_CACHE[key]


def _perm_cols(Dm):
    """Column permutation for the K/Q DoubleRow band layout.
    Chunk m=2g+t, partition p=32*b4+r  ->  original feature
    (4g+b4)*64 + t*32 + r."""
    perm = np.empty(Dm, np.int64)
    for m in range(Dm // P):
        g, t = m // 2, m % 2
        for p in range(P):
            b4, r = p // 32, p % 32
            perm[m * P + p] = (4 * g + b4) * 64 + t * 32 + r
    return perm


def _dr_weight(w, scale=1.0):
    """[D, N] -> (128, D/256, 2, N) fp8: [r, p, t, c] = w[(2p+t)*128+r, c]."""
    Dm = w.shape[0]
    f8 = ml_dtypes.float8_e4m3
    return np.ascontiguousarray(
        (w.reshape(Dm // 256, 2, P, w.shape[1]) * scale)
        .transpose(2, 0, 1, 3)).astype(f8)


def make_in_maps(inputs, n_cores):
    x = np.asarray(inputs["x"], np.float32)
    Bq, Sq, Dq = x.shape
    Qtok = Sq * Bq // n_cores
    bf = ml_dtypes.bfloat16
    perm = _perm_cols(Dq)
    wk = np.asarray(inputs["Wk"], np.float32)[:, perm]
    wq = np.asarray(inputs["Wq"], np.float32)[:, perm]
    shared = {
        "g1": np.asarray(inputs["ln1_g"], np.float32),
        "be1": np.asarray(inputs["ln1_b"], np.float32),
        "g2": np.asarray(inputs["ln2_g"], np.float32),
        "be2": np.asarray(inputs["ln2_b"], np.float32),
        "wk8": _dr_weight(wk),
        "wq8": _dr_weight(wq),
        "wv8": _dr_weight(np.asarray(inputs["Wv"], np.float32)),
        "wo8": _dr_weight(np.asarray(inputs["Wo"], np.float32)),
        "w1h": np.asarray(inputs["W1"], np.float32).astype(bf),
        "w2h": np.asarray(inputs["W2"], np.float32).astype(bf),
        "bkp": np.asarray(inputs["bk"], np.float32)[perm],
        "bqp": np.asarray(inputs["bq"], np.float32)[perm],
        "bv": np.asarray(inputs["bv"], np.float32),
        "bo": np.asarray(inputs["bo"], np.float32),
        "b1": np.asarray(inputs["b1"], np.float32),
        "b2": np.asarray(inputs["b2"], np.float32),
        "ones16": np.ones((P, 1), bf),
        "ident16": np.eye(P, dtype=bf),
    }
    in_maps = []
    for c in range(n_cores):
        b = c // (n_cores // Bq)
        qoff = (c % (n_cores // Bq)) * Qtok
        m = dict(shared)
        xrot = np.concatenate([x[b, qoff:], x[b, :qoff]], axis=0)
        m["xT"] = np.ascontiguousarray(xrot.T)
        in_maps.append(m)
    return in_maps, Qtok


def kernel(**inputs):
    x = np.asarray(inputs["x"], np.float32)
    Bq, Sq, Dq = x.shape
    in_maps, Qtok = make_in_maps(inputs, N_CORES)
    nc = _get_nc(Sq, Qtok, Dq, H, MLP, N_CORES)
    res = run_bass_kernel_spmd(nc, in_maps, core_ids=list(range(N_CORES)))
    out = np.empty((Bq, Sq, Dq), np.float32)
    per_b = N_CORES // Bq
    for c in range(N_CORES):
        b = c // per_b
        qoff = (c % per_b) * Qtok
        out[b, qoff:qoff + Qtok, :] = res.results[c]["yT"].T
    return out
